# revision 2
# baseline (speedup 1.0000x reference)
"""Trainium2 Bass kernel for nn_BiSVM: out[b,o] = diag(L @ x[b] @ R).

Math: out[b,o] = sum_{i,j} L[o,i] * x[b,i,j] * R[j,o]
  step 1 (TensorE): lx[o,j] = sum_i LT[i,o]^T @ x[b,i,j]   (LT = L^T, stationary)
  step 2 (VectorE): out[b,o] = sum_j lx[o,j] * RT[o,j]      (RT = R^T, fused
          multiply+reduce via scalar_tensor_tensor accum_out)

Sharding: data-parallel over batch, 8 batches per core on 8 NeuronCores;
L/R replicated. x and L are cast to fp16 on the host (PE matmul runs fp16 at
full rate, 1 cycle/row; products are exact, accumulation is fp32 in PSUM —
end-to-end error ~3e-4 relative to the fp32 reference). R stays fp32 in the
vector-engine reduction.

Self-contained: hardcodes shapes B=64, I=O=J=1024, 8 cores.
"""

import numpy as np

import concourse.bacc as bacc
import concourse.mybir as mybir
import concourse.tile as tile
from concourse.bass_utils import run_bass_kernel_spmd

B, I, O, J = 64, 1024, 1024, 1024
NCORES = 8
BPC = B // NCORES          # batches per core
BBLK = 2                   # batches per SBUF-resident block
NBLK = BPC // BBLK
NOT = O // 128             # o-tiles
NIT = I // 128             # i-tiles (contraction)
NJC = J // 512             # j-chunks (psum bank width)

f16 = mybir.dt.float16
f32 = mybir.dt.float32


def build_nc():
    nc = bacc.Bacc("TRN2", target_bir_lowering=False, debug=False)
    x_d = nc.dram_tensor("x", [BPC, I, J], f16, kind="ExternalInput")
    lt_d = nc.dram_tensor("lt", [I, O], f16, kind="ExternalInput")
    rt_d = nc.dram_tensor("rt", [O, J], f32, kind="ExternalInput")
    # out_sb layout: [o_within_tile(128), ot(8) * b(8)] ; host reassembles
    out_d = nc.dram_tensor("out", [128, NOT * BPC], f32, kind="ExternalOutput")

    with tile.TileContext(nc) as tc:
        with (
            tc.tile_pool(name="w", bufs=1) as wpool,
            tc.tile_pool(name="xp", bufs=2 * BBLK) as xpool,
            tc.tile_pool(name="sc", bufs=4) as spool,
            tc.tile_pool(name="ps", bufs=4, space="PSUM") as pspool,
        ):
            lt_sb = wpool.tile([128, NIT, O], f16, name="lt_sb")
            nc.sync.dma_start(
                lt_sb[:], lt_d.ap().rearrange("(t p) o -> p t o", p=128))
            rt_sb = wpool.tile([128, NOT, J], f32, name="rt_sb")
            nc.sync.dma_start(
                rt_sb[:], rt_d.ap().rearrange("(t p) j -> p t j", p=128))
            out_sb = wpool.tile([128, NOT * BPC], f32, name="out_sb")

            for blk in range(NBLK):
                xts = []
                for bb in range(BBLK):
                    b = blk * BBLK + bb
                    xt = xpool.tile([128, NIT, J], f16,
                                    name=f"x_{b}", tag="xt")
                    nc.sync.dma_start(
                        xt[:],
                        x_d.ap()[b, :, :].rearrange("(t p) j -> p t j", p=128))
                    xts.append(xt)
                for ot in range(NOT):
                    pss = [
                        pspool.tile([128, J], f32,
                                    name=f"ps_{blk}_{ot}_{s}", tag="ps")
                        for s in range(BBLK)
                    ]
                    for it in range(NIT):
                        lhsT = lt_sb[:, it, ot * 128:(ot + 1) * 128]
                        for bb in range(BBLK):
                            for jc in range(NJC):
                                nc.tensor.matmul(
                                    pss[bb][:, jc * 512:(jc + 1) * 512],
                                    lhsT,
                                    xts[bb][:, it, jc * 512:(jc + 1) * 512],
                                    start=(it == 0),
                                    stop=(it == NIT - 1),
                                )
                    for bb in range(BBLK):
                        b = blk * BBLK + bb
                        sc0 = spool.tile([128, J], f32,
                                         name=f"sc0_{b}_{ot}", tag="sc")
                        col = ot * BPC + b
                        # out = (ps * 1.0) * rt ; accum_out = sum_j(out)
                        nc.vector.scalar_tensor_tensor(
                            out=sc0[:],
                            in0=pss[bb][:],
                            scalar=1.0,
                            in1=rt_sb[:, ot, :],
                            op0=mybir.AluOpType.mult,
                            op1=mybir.AluOpType.mult,
                            accum_out=out_sb[:, col:col + 1],
                        )
            nc.sync.dma_start(out_d.ap(), out_sb[:])
    nc.compile()
    return nc


_NC_CACHE = []


def _get_nc():
    if not _NC_CACHE:
        _NC_CACHE.append(build_nc())
    return _NC_CACHE[0]


def make_in_maps(x: np.ndarray, L: np.ndarray, R: np.ndarray):
    xx = np.ascontiguousarray(x).astype(np.float16)
    lt = np.ascontiguousarray(L.T).astype(np.float16)
    rt = np.ascontiguousarray(R.T).astype(np.float32)
    return [
        {"x": xx[c * BPC:(c + 1) * BPC], "lt": lt, "rt": rt}
        for c in range(NCORES)
    ]


def assemble(results) -> np.ndarray:
    out = np.empty((B, O), np.float32)
    for c in range(NCORES):
        oc = results[c]["out"]                      # [128, NOT*BPC]
        t = oc.reshape(128, NOT, BPC)               # [p, ot, b]
        out[c * BPC:(c + 1) * BPC] = t.transpose(2, 1, 0).reshape(BPC, O)
    return out


def kernel(x: np.ndarray, L: np.ndarray, R: np.ndarray) -> np.ndarray:
    nc = _get_nc()
    res = run_bass_kernel_spmd(nc, make_in_maps(x, L, R),
                               core_ids=list(range(NCORES)))
    return assemble(res.results)


# revision 5
# speedup vs baseline: 148.6131x; 148.6131x over previous
"""Trainium2 Bass kernel for nn_BiSVM: out[b,o] = diag(L @ x[b] @ R).

Math: out[b,o] = sum_{i,j} L[o,i] * x[b,i,j] * R[j,o]
  step 1 (TensorE): lx[o,j] = sum_i LT[i,o]^T @ x[b,i,j]   (LT = L^T, stationary)
  step 2 (VectorE): out[b,o] = sum_j lx[o,j] * RT[o,j]      (RT = R^T, fused
          multiply+reduce via scalar_tensor_tensor accum_out)

Sharding: data-parallel over batch, 8 batches per core on 8 NeuronCores;
L/R replicated. x and L are cast to fp16 on the host (PE matmul runs fp16 at
full rate, 1 cycle/row; products are exact, accumulation is fp32 in PSUM —
end-to-end error ~3e-4 relative to the fp32 reference). R stays fp32 in the
vector-engine reduction.

Self-contained: hardcodes shapes B=64, I=O=J=1024, 8 cores.
"""

import numpy as np

import concourse.bacc as bacc
import concourse.mybir as mybir
import concourse.tile as tile
from concourse.bass_utils import run_bass_kernel_spmd

B, I, O, J = 64, 1024, 1024, 1024
NCORES = 8
BPC = B // NCORES          # batches per core
BBLK = 2                   # batches per SBUF-resident block
NBLK = BPC // BBLK
NOT = O // 128             # o-tiles
NIT = I // 128             # i-tiles (contraction)
NJC = J // 512             # j-chunks (psum bank width)

f16 = mybir.dt.float16
f32 = mybir.dt.float32


def build_nc(reps: int | None = None):
    nc = bacc.Bacc("TRN2", target_bir_lowering=False, debug=False)
    x_d = nc.dram_tensor("x", [BPC, I, J], f16, kind="ExternalInput")
    lt_d = nc.dram_tensor("lt", [I, O], f16, kind="ExternalInput")
    rt_d = nc.dram_tensor("rt", [O, J], f32, kind="ExternalInput")
    # out_sb layout: [o_within_tile(128), ot(8) * b(8)] ; host reassembles
    out_d = nc.dram_tensor("out", [128, NOT * BPC], f32, kind="ExternalOutput")

    import contextlib

    def body(tc, wpool, xpool, spool, pspool):
            lt_sb = wpool.tile([128, NIT, O], f16, name="lt_sb")
            for lts in range(NIT):
                nc.sync.dma_start(
                    lt_sb[:, lts:lts + 1, :],
                    lt_d.ap()[lts * 128:(lts + 1) * 128, :]
                    .rearrange("(t p) o -> p t o", p=128))
            rt_sb = wpool.tile([128, NOT, J], f32, name="rt_sb")
            nc.sync.dma_start(
                rt_sb[:], rt_d.ap().rearrange("(t p) j -> p t j", p=128))
            out_sb = wpool.tile([128, NOT * BPC], f32, name="out_sb")

            for blk in range(NBLK):
                xts = []
                for bb in range(BBLK):
                    b = blk * BBLK + bb
                    xt = xpool.tile([128, NIT, J], f16,
                                    name=f"x_{b}", tag="xt")
                    for sp in range(NIT):
                        nc.sync.dma_start(
                            xt[:, sp:sp + 1, :],
                            x_d.ap()[b, sp * 128:(sp + 1) * 128, :]
                            .rearrange("(t p) j -> p t j", p=128))
                    xts.append(xt)
                for ot in range(NOT):
                    pss = [
                        pspool.tile([128, J], f32,
                                    name=f"ps_{blk}_{ot}_{s}", tag="ps")
                        for s in range(BBLK)
                    ]
                    for it in range(NIT):
                        lhsT = lt_sb[:, it, ot * 128:(ot + 1) * 128]
                        for bb in range(BBLK):
                            for jc in range(NJC):
                                nc.tensor.matmul(
                                    pss[bb][:, jc * 512:(jc + 1) * 512],
                                    lhsT,
                                    xts[bb][:, it, jc * 512:(jc + 1) * 512],
                                    start=(it == 0),
                                    stop=(it == NIT - 1),
                                )
                    for bb in range(BBLK):
                        b = blk * BBLK + bb
                        sc0 = spool.tile([128, J], f32,
                                         name=f"sc0_{b}_{ot}", tag="sc")
                        col = ot * BPC + b
                        # out = (ps * 1.0) * rt ; accum_out = sum_j(out)
                        nc.vector.scalar_tensor_tensor(
                            out=sc0[:],
                            in0=pss[bb][:],
                            scalar=1.0,
                            in1=rt_sb[:, ot, :],
                            op0=mybir.AluOpType.mult,
                            op1=mybir.AluOpType.mult,
                            accum_out=out_sb[:, col:col + 1],
                        )
            nc.sync.dma_start(out_d.ap(), out_sb[:])

    with tile.TileContext(nc) as tc:
        with (
            tc.tile_pool(name="w", bufs=1) as wpool,
            tc.tile_pool(name="xp", bufs=2 * BBLK) as xpool,
            tc.tile_pool(name="sc", bufs=4) as spool,
            tc.tile_pool(name="ps", bufs=4, space="PSUM") as pspool,
        ):
            loop = (tc.For_i(0, reps, 1) if reps is not None
                    else contextlib.nullcontext())
            with loop:
                body(tc, wpool, xpool, spool, pspool)
    nc.compile()
    return nc


_NC_CACHE = []


def _get_nc():
    if not _NC_CACHE:
        _NC_CACHE.append(build_nc())
    return _NC_CACHE[0]


def make_in_maps(x: np.ndarray, L: np.ndarray, R: np.ndarray):
    xx = np.ascontiguousarray(x).astype(np.float16)
    lt = np.ascontiguousarray(L.T).astype(np.float16)
    rt = np.ascontiguousarray(R.T).astype(np.float32)
    return [
        {"x": xx[c * BPC:(c + 1) * BPC], "lt": lt, "rt": rt}
        for c in range(NCORES)
    ]


def assemble(results) -> np.ndarray:
    out = np.empty((B, O), np.float32)
    for c in range(NCORES):
        oc = results[c]["out"]                      # [128, NOT*BPC]
        t = oc.reshape(128, NOT, BPC)               # [p, ot, b]
        out[c * BPC:(c + 1) * BPC] = t.transpose(2, 1, 0).reshape(BPC, O)
    return out


def kernel(x: np.ndarray, L: np.ndarray, R: np.ndarray) -> np.ndarray:
    nc = _get_nc()
    res = run_bass_kernel_spmd(nc, make_in_maps(x, L, R),
                               core_ids=list(range(NCORES)))
    return assemble(res.results)



# revision 6
# speedup vs baseline: 148.6897x; 1.0005x over previous
"""Trainium2 Bass kernel for nn_BiSVM: out[b,o] = diag(L @ x[b] @ R).

Math: out[b,o] = sum_{i,j} L[o,i] * x[b,i,j] * R[j,o]
  step 1 (TensorE): lx[o,j] = sum_i LT[i,o]^T @ x[b,i,j]   (LT = L^T, stationary)
  step 2 (VectorE): out[b,o] = sum_j lx[o,j] * RT[o,j]      (RT = R^T, fused
          multiply+reduce via scalar_tensor_tensor accum_out)

Sharding: data-parallel over batch, 8 batches per core on 8 NeuronCores;
L/R replicated. x and L are cast to fp16 on the host (PE matmul runs fp16 at
full rate, 1 cycle/row; products are exact, accumulation is fp32 in PSUM —
end-to-end error ~3e-4 relative to the fp32 reference). R stays fp32 in the
vector-engine reduction.

Self-contained: hardcodes shapes B=64, I=O=J=1024, 8 cores.
"""

import numpy as np

import concourse.bacc as bacc
import concourse.mybir as mybir
import concourse.tile as tile
from concourse.bass_utils import run_bass_kernel_spmd

B, I, O, J = 64, 1024, 1024, 1024
NCORES = 8
BPC = B // NCORES          # batches per core
BBLK = 2                   # batches per SBUF-resident block
NBLK = BPC // BBLK
NOT = O // 128             # o-tiles
NIT = I // 128             # i-tiles (contraction)
NJC = J // 512             # j-chunks (psum bank width)

f16 = mybir.dt.float16
f32 = mybir.dt.float32


def build_nc(reps: int | None = None):
    nc = bacc.Bacc("TRN2", target_bir_lowering=False, debug=False)
    x_d = nc.dram_tensor("x", [BPC, I, J], f16, kind="ExternalInput")
    lt_d = nc.dram_tensor("lt", [I, O], f16, kind="ExternalInput")
    rt_d = nc.dram_tensor("rt", [O, J], f32, kind="ExternalInput")
    # out_sb layout: [o_within_tile(128), ot(8) * b(8)] ; host reassembles
    out_d = nc.dram_tensor("out", [128, NOT * BPC], f32, kind="ExternalOutput")

    import contextlib

    def body(tc, wpool, xpool, spool, pspool):
            lt_sb = wpool.tile([128, NIT, O], f16, name="lt_sb")
            for lts in range(NIT):
                nc.sync.dma_start(
                    lt_sb[:, lts:lts + 1, :],
                    lt_d.ap()[lts * 128:(lts + 1) * 128, :]
                    .rearrange("(t p) o -> p t o", p=128))
            rt_sb = wpool.tile([128, NOT, J], f32, name="rt_sb")
            out_sb = wpool.tile([128, NOT * BPC], f32, name="out_sb")

            for blk in range(NBLK):
                xts = []
                for bb in range(BBLK):
                    b = blk * BBLK + bb
                    xt = xpool.tile([128, NIT, J], f16,
                                    name=f"x_{b}", tag="xt")
                    for sp in range(NIT):
                        nc.sync.dma_start(
                            xt[:, sp:sp + 1, :],
                            x_d.ap()[b, sp * 128:(sp + 1) * 128, :]
                            .rearrange("(t p) j -> p t j", p=128))
                    xts.append(xt)
                if blk == 0:
                    nc.sync.dma_start(
                        rt_sb[:],
                        rt_d.ap().rearrange("(t p) j -> p t j", p=128))
                for ot in range(NOT):
                    pss = [
                        pspool.tile([128, J], f32,
                                    name=f"ps_{blk}_{ot}_{s}", tag="ps")
                        for s in range(BBLK)
                    ]
                    for it in range(NIT):
                        lhsT = lt_sb[:, it, ot * 128:(ot + 1) * 128]
                        for bb in range(BBLK):
                            for jc in range(NJC):
                                nc.tensor.matmul(
                                    pss[bb][:, jc * 512:(jc + 1) * 512],
                                    lhsT,
                                    xts[bb][:, it, jc * 512:(jc + 1) * 512],
                                    start=(it == 0),
                                    stop=(it == NIT - 1),
                                )
                    for bb in range(BBLK):
                        b = blk * BBLK + bb
                        sc0 = spool.tile([128, J], f32,
                                         name=f"sc0_{b}_{ot}", tag="sc")
                        col = ot * BPC + b
                        # out = (ps * 1.0) * rt ; accum_out = sum_j(out)
                        nc.vector.scalar_tensor_tensor(
                            out=sc0[:],
                            in0=pss[bb][:],
                            scalar=1.0,
                            in1=rt_sb[:, ot, :],
                            op0=mybir.AluOpType.mult,
                            op1=mybir.AluOpType.mult,
                            accum_out=out_sb[:, col:col + 1],
                        )
            nc.sync.dma_start(out_d.ap(), out_sb[:])

    with tile.TileContext(nc) as tc:
        with (
            tc.tile_pool(name="w", bufs=1) as wpool,
            tc.tile_pool(name="xp", bufs=2 * BBLK) as xpool,
            tc.tile_pool(name="sc", bufs=4) as spool,
            tc.tile_pool(name="ps", bufs=4, space="PSUM") as pspool,
        ):
            loop = (tc.For_i(0, reps, 1) if reps is not None
                    else contextlib.nullcontext())
            with loop:
                body(tc, wpool, xpool, spool, pspool)
    nc.compile()
    return nc


_NC_CACHE = []


def _get_nc():
    if not _NC_CACHE:
        _NC_CACHE.append(build_nc())
    return _NC_CACHE[0]


def make_in_maps(x: np.ndarray, L: np.ndarray, R: np.ndarray):
    xx = np.ascontiguousarray(x).astype(np.float16)
    lt = np.ascontiguousarray(L.T).astype(np.float16)
    rt = np.ascontiguousarray(R.T).astype(np.float32)
    return [
        {"x": xx[c * BPC:(c + 1) * BPC], "lt": lt, "rt": rt}
        for c in range(NCORES)
    ]


def assemble(results) -> np.ndarray:
    out = np.empty((B, O), np.float32)
    for c in range(NCORES):
        oc = results[c]["out"]                      # [128, NOT*BPC]
        t = oc.reshape(128, NOT, BPC)               # [p, ot, b]
        out[c * BPC:(c + 1) * BPC] = t.transpose(2, 1, 0).reshape(BPC, O)
    return out


def kernel(x: np.ndarray, L: np.ndarray, R: np.ndarray) -> np.ndarray:
    nc = _get_nc()
    res = run_bass_kernel_spmd(nc, make_in_maps(x, L, R),
                               core_ids=list(range(NCORES)))
    return assemble(res.results)

